# revision 1
# baseline (speedup 1.0000x reference)
"""Trainium2 Bass kernel for nn_Basic_MPNN (gnn_message_passing).

Math (per batch b):
  m1 = node @ W1 + b1                  [N, MID]   (receiver side, axis 2)
  m2 = node @ W2 + b2                  [N, MID]   (sender side, axis 1)
  me = edge @ We + be                  [N, N, MID]
  mg = graph @ Wg + bg                 [MID]
  msgs[j,i,:] = m1[i] + m2[j] + me[j,i] + mg
  M[i,:] = max_j where(adj[j,i], msgs[j,i,:], -1e6)
  out = relu(node @ Wo1 + bo1 + M @ Wo2 + bo2)

Sharding: 8 cores = (4 batches) x (2 receiver halves of 256).

Per-core device algorithm (roofline = streaming the 64 MiB edge slice):
  cT[mid,i] = (m1[i] + mg + b1+be+bg)^T computed once (fp32).
  For each sender j and receiver block: DMA edge tile [128 i, 128 d]
  (fp32->fp16 cast in the DMA), PE-transpose to [d, i]; per 4-j batch one
  N=512 fp16 matmul with stationary We producing meT slots [mid, j*128+i]
  in PSUM; then per j a rank-2 matmul accumulates
    adj01[j,i] * m2[j,mid] + (1-adj01[j,i]) * (-60000)
  which applies mask and sender term exactly (products with the 0/1
  gate are exact; no large-constant rounding touches live values).
  DVE reduce_max folds 8 slots at a time, then two more 8-way levels,
  all in [mid, i] layout.
  Finalize: M = max(Mraw + cT, -1e6); out = relu(noderT.T@Wo1 + M.T'@Wo2 + b).

Rank-2 row-group placement: all rank-2 matmuls of sender-group g share PE
row-group k = g // 16 (two adjacent row-grouped matmuls with *different*
tile_position inside an open PSUM accumulation group crash the HW --
verified experimentally; same tile_position back-to-back is fine, and a
full-K matmul between them is fine). The build asserts the final PE
schedule has no unsafe adjacency.
"""

import os
import sys

for _p in (
    "/root/.axon_site",
    "/root/.axon_site/_ro/trn_rl_repo",
    "/root/.axon_site/_ro/pypackages",
    "/opt/trn_rl_repo",
    "/opt/pypackages",
):
    if os.path.isdir(_p) and _p not in sys.path:
        sys.path.append(_p)

import numpy as np  # noqa: E402

import concourse.bass as bass  # noqa: E402
import concourse.tile as tile  # noqa: E402
from concourse import bacc, masks, mybir  # noqa: E402
from concourse.bass_utils import run_bass_kernel_spmd  # noqa: E402

F32 = mybir.dt.float32
F16 = mybir.dt.float16
I32 = mybir.dt.int32

B, N, D, MID, OUT = 4, 512, 128, 128, 128
NCORES = 8
IH = N // 2  # receivers per core
JG = 8       # senders per j-group
NG = N // JG  # 64 j-groups
L2W = 8      # groups per level-2 reduce
MASK_NEG = -60000.0  # < any valid msg value; fp16-representable
BIG_NUMBER = 1.0e6


def _k_of_group(g):
    # row-group for sender-group g; constant across 16-group spans so
    # adjacent rank-2 matmuls share tile_position almost everywhere
    return g // 16


def _u_of_j(j):
    return j % 128


def _build_program(repeat=1):
    nc = bacc.Bacc(
        "TRN2", target_bir_lowering=False, debug=False, num_devices=NCORES
    )

    edge = nc.dram_tensor("edge", [N, IH, D], F32, kind="ExternalInput").ap()
    nodeT_d = nc.dram_tensor("nodeT", [D, N], F32, kind="ExternalInput").ap()
    noderT_d = nc.dram_tensor("noderT", [D, IH], F32, kind="ExternalInput").ap()
    graph = nc.dram_tensor("graph", [1, D], F32, kind="ExternalInput").ap()
    adj = nc.dram_tensor("adj", [N, IH], I32, kind="ExternalInput").ap()
    wpack_d = nc.dram_tensor("wpack", [D, 5 * MID], F32, kind="ExternalInput").ap()
    bpack_d = nc.dram_tensor("bpack", [1, 6 * MID], F32, kind="ExternalInput").ap()
    we_d = nc.dram_tensor("We", [D, MID], F32, kind="ExternalInput").ap()
    out_d = nc.dram_tensor("out", [IH, OUT], F32, kind="ExternalOutput").ap()

    with (
        tile.TileContext(nc) as tc,
        tc.tile_pool(name="persist", bufs=1) as pp,
        tc.tile_pool(name="setup_sb", bufs=1) as ssb,
        tc.tile_pool(name="accum", bufs=1) as accp,
        tc.tile_pool(name="edge", bufs=8) as ep,
        tc.tile_pool(name="tf", bufs=8) as tfp,
        tc.tile_pool(name="ps8", bufs=3, space="PSUM") as ps8p,
        tc.tile_pool(name="psT", bufs=2, space="PSUM") as psTp,
    ):
        if True:
            # ---------------- adjacency in rank-2 rhs layout ----------------
            # adjn[u, k, i] = adj[128k+u, i]
            adjn = ssb.tile([128, 4 * IH], I32)
            nc.sync.dma_start(
                adjn[:], adj.rearrange("(k u) i -> u k i", k=4)
            )
            a01_32 = ssb.tile([128, 4 * IH], F32)
            nc.vector.tensor_copy(a01_32[:], adjn[:])
            a01 = ssb.tile([128, 4 * IH], F16)
            nc.vector.tensor_copy(a01[:], a01_32[:])
            inv01 = ssb.tile([128, 4 * IH], F16)
            nc.vector.tensor_scalar(
                inv01[:], a01_32[:], -1.0, 1.0,
                op0=mybir.AluOpType.mult, op1=mybir.AluOpType.add,
            )
            # adjr2[32k+0, u*256 + ib*128 + il] = adj01[j, ib*128+il]
            adjr2 = pp.tile([128, 128 * IH], F16)
            for k in range(4):
                nc.sync.dma_start(
                    adjr2[32 * k:32 * k + 1, :], a01[:, k * IH:(k + 1) * IH]
                )
                nc.scalar.dma_start(
                    adjr2[32 * k + 1:32 * k + 2, :],
                    inv01[:, k * IH:(k + 1) * IH],
                )
            # ---------------- constants & weights ----------------
            ident16 = pp.tile([128, 128], F16)
            masks.make_identity(nc, ident16[:])
            ones32 = pp.tile([1, 256], F32)
            nc.vector.memset(ones32[:], 1.0)

            # node features first: they gate the m2 -> m2r2 chain
            nodeT = pp.tile([D, N], F32)
            nc.sync.dma_start(nodeT[:], nodeT_d[:, :])
            noderT = pp.tile([D, IH], F32)
            nc.scalar.dma_start(noderT[:], noderT_d[:, :])
            wpack = pp.tile([D, 5 * MID], F32)
            nc.sync.dma_start(wpack[:], wpack_d[:, :])
            bpack = pp.tile([1, 6 * MID], F32)
            nc.scalar.dma_start(bpack[:], bpack_d[:, :])
            wsb = {
                w: wpack[:, i * MID:(i + 1) * MID]
                for i, w in enumerate(("W2", "W1", "Wg", "Wo1", "Wo2"))
            }
            bsb = {
                b: bpack[:, i * MID:(i + 1) * MID]
                for i, b in enumerate(("b1", "b2", "be", "bg", "bo1", "bo2"))
            }
            we16 = pp.tile([D, MID], F16)
            nc.gpsimd.dma_start(we16[:], we_d[:, :])  # cast f32->f16

            # ---------------- m2 in rank-2 lhsT layout ----------------
            # m2r2[32k+0, u*128+mid] = m2[j, mid] (f16), j = 128k + u;
            # m2r2[32k+1, ...] = MASK_NEG
            m2r2 = pp.tile([128, 128 * MID], F16)
            neg_sb = ssb.tile([128, 512], F16)
            nc.vector.memset(neg_sb[:], MASK_NEG)
            m2f16 = ssb.tile([128, 4 * MID], F16)
            # nodeT columns j = 128k + u
            for k in range(4):
                ps_m2 = psTp.tile([128, MID], F32, tag="pT")
                nc.tensor.matmul(
                    ps_m2[:],
                    lhsT=nodeT[:, k * 128:(k + 1) * 128],
                    rhs=wsb["W2"], start=True, stop=False,
                )
                nc.tensor.matmul(
                    ps_m2[:], lhsT=ones32[:, 0:128], rhs=bsb["b2"],
                    start=False, stop=True,
                )
                nc.scalar.copy(m2f16[:, k * MID:(k + 1) * MID], ps_m2[:])
            for k in range(4):
                nc.sync.dma_start(
                    m2r2[32 * k:32 * k + 1, :],
                    m2f16[:, k * MID:(k + 1) * MID],
                )
                nc.scalar.dma_start(
                    m2r2[32 * k + 1:32 * k + 2, :], neg_sb[0:32, :]
                )


            # r = mg + b1 + be + bg ; bso = bo1 + bo2
            gT = ssb.tile([D, 1], F32)
            nc.sync.dma_start(gT[:], graph[0:1, :])
            ps_mg = psTp.tile([1, MID], F32, tag="pT")
            nc.tensor.matmul(ps_mg[:], lhsT=gT[:], rhs=wsb["Wg"], start=True, stop=True)
            r_sb = pp.tile([1, MID], F32)
            nc.scalar.copy(r_sb[:], ps_mg[:])
            nc.vector.tensor_add(r_sb[:], r_sb[:], bsb["b1"])
            nc.vector.tensor_add(r_sb[:], r_sb[:], bsb["be"])
            nc.vector.tensor_add(r_sb[:], r_sb[:], bsb["bg"])
            bso = pp.tile([1, MID], F32)
            nc.vector.tensor_add(bso[:], bsb["bo1"], bsb["bo2"])

            # ---------------- cT[mid, i] = (m1 + r)^T ----------------
            ps_cT = psTp.tile([128, IH], F32, name="ps_cT", tag="pT")
            nc.tensor.matmul(
                ps_cT[:], lhsT=wsb["W1"][:], rhs=noderT[:], start=True, stop=False
            )
            nc.tensor.matmul(
                ps_cT[:], lhsT=r_sb[:], rhs=ones32[:], start=False, stop=True
            )
            cT_sb = pp.tile([128, IH], F32)
            nc.scalar.copy(cT_sb[:], ps_cT[:])

        # ---------------- main streaming loop ----------------
        redbuf = [None, None]
        l2buf = [None, None]
        if True:
            for ib in range(2):
                redbuf[ib] = accp.tile([128, 2 * L2W * MID], F32, name=f"red{ib}")
                l2buf[ib] = accp.tile([128, (NG // L2W) * MID], F32, name=f"l2{ib}")

            if True:
                # Software pipeline: per unit (g, ib) emit the transposes and
                # PSUM->SBUF copies; the We-matmuls + rank-2 + reduce for a
                # unit are emitted one unit later so the PE never head-of-line
                # blocks on the Activation copy of its own transposes.
                def emit_mm_reduce(st):
                    g, ib, tfs = st
                    k = _k_of_group(g)
                    ps8 = ps8p.tile([128, JG * MID], F32, tag="ps8")
                    for half in range(2):
                        nc.tensor.matmul(
                            ps8[:, half * 512:(half + 1) * 512],
                            lhsT=we16[:], rhs=tfs[half][:],
                            start=True, stop=False,
                        )
                        for q in range(4):
                            jl = half * 4 + q
                            j = g * JG + jl
                            u = _u_of_j(j)
                            nc.tensor.matmul(
                                ps8[:, jl * MID:(jl + 1) * MID],
                                lhsT=m2r2[32 * k:32 * k + 2,
                                          u * 128:(u + 1) * 128],
                                rhs=adjr2[32 * k:32 * k + 2,
                                          u * 256 + ib * 128:u * 256 + ib * 128 + 128],
                                start=False, stop=(q == 3),
                                tile_position=(32 * k, 0),
                            )
                    slot = g % (2 * L2W)
                    nc.vector.tensor_reduce(
                        redbuf[ib][:, slot * MID:(slot + 1) * MID],
                        ps8[:].rearrange("p (s m) -> p m s", s=JG),
                        axis=mybir.AxisListType.X,
                        op=mybir.AluOpType.max,
                    )
                    if g % L2W == L2W - 1:
                        par = (g // L2W) % 2
                        nc.vector.tensor_reduce(
                            l2buf[ib][:, (g // L2W) * MID:(g // L2W + 1) * MID],
                            redbuf[ib][:, par * L2W * MID:(par + 1) * L2W * MID]
                            .rearrange("p (s m) -> p m s", s=L2W),
                            axis=mybir.AxisListType.X,
                            op=mybir.AluOpType.max,
                        )

                stash = []
                e_t = None
                for g in range(repeat * NG):
                    g = g % NG
                    e_t = ep.tile([128, JG * 2 * D], F16, tag="e")
                    nc.gpsimd.dma_start(
                        e_t[:],
                        edge[g * JG:(g + 1) * JG]
                        .rearrange("j (ib p) d -> p j ib d", p=128),
                    )
                    for ib in range(2):
                        tfs = []
                        for half in range(2):
                            pT = psTp.tile([128, 512], F16, tag="pT")
                            for q in range(4):
                                jl = half * 4 + q
                                nc.tensor.transpose(
                                    pT[:, q * 128:(q + 1) * 128],
                                    e_t[:, (jl * 2 + ib) * D:(jl * 2 + ib + 1) * D],
                                    ident16[:],
                                )
                            tf = tfp.tile([128, 512], F16, tag="tf")
                            nc.scalar.copy(tf[:], pT[:])
                            tfs.append(tf)
                        stash.append((g, ib, tfs))
                        if len(stash) > 1:
                            emit_mm_reduce(stash.pop(0))
                while stash:
                    emit_mm_reduce(stash.pop(0))

            # ---------------- finalize ----------------
            with (
                tc.tile_pool(name="fin_sb", bufs=2) as fsb,
            ):
                fps = psTp
                for ib in range(2):
                    mraw = fsb.tile([128, MID], F32, tag="mraw")
                    nc.vector.tensor_reduce(
                        mraw[:],
                        l2buf[ib][:].rearrange("p (s m) -> p m s", s=NG // L2W),
                        axis=mybir.AxisListType.X,
                        op=mybir.AluOpType.max,
                    )
                    # msgs^T [mid, i] = max(mraw + cT, -1e6)
                    msgs = fsb.tile([128, MID], F32, tag="msgs")
                    nc.vector.tensor_add(
                        msgs[:], mraw[:], cT_sb[:, ib * MID:(ib + 1) * MID]
                    )
                    nc.vector.tensor_scalar_max(msgs[:], msgs[:], -BIG_NUMBER)
                    ps_h = fps.tile([128, OUT], F32, tag="pT")
                    nc.tensor.matmul(
                        ps_h[:], lhsT=msgs[:], rhs=wsb["Wo2"],
                        start=True, stop=False,
                    )
                    nc.tensor.matmul(
                        ps_h[:], lhsT=noderT[:, ib * 128:(ib + 1) * 128],
                        rhs=wsb["Wo1"], start=False, stop=False,
                    )
                    nc.tensor.matmul(
                        ps_h[:], lhsT=ones32[:, 0:128], rhs=bso[:],
                        start=False, stop=True,
                    )
                    o_sb = fsb.tile([128, OUT], F32, tag="osb")
                    nc.scalar.activation(
                        o_sb[:], ps_h[:], mybir.ActivationFunctionType.Relu
                    )
                    nc.sync.dma_start(out_d[ib * 128:(ib + 1) * 128, :], o_sb[:])

    nc.finalize()
    _assert_safe_pe_schedule(nc)
    return nc


def _assert_safe_pe_schedule(nc):
    """No two adjacent sub-tile (row-grouped) matmuls with different
    tile_position in the final PE stream (HW crash pattern)."""
    prev = None
    for func in nc.m.functions:
        for block in func.blocks:
            for inst in block.instructions:
                if not isinstance(inst, mybir.InstMatmult):
                    continue
                rows = inst.tile_size[0] if inst.tile_size else 128
                sub = rows < 128
                cur = (sub, tuple(inst.tile_position or (0, 0)))
                if (
                    prev is not None
                    and prev[0] and sub
                    and prev[1] != cur[1]
                ):
                    raise AssertionError(
                        f"unsafe adjacent row-grouped matmuls: {prev} -> {cur}"
                    )
                prev = cur
    return True


_CACHED = {}


def _get_program():
    if "nc" not in _CACHED:
        _CACHED["nc"] = _build_program()
    return _CACHED["nc"]


def kernel(**inputs) -> np.ndarray:
    nc = _get_program()

    def f32(x):
        return np.ascontiguousarray(np.asarray(x, dtype=np.float32))

    node_fts = f32(inputs["node_fts"])
    edge_fts = f32(inputs["edge_fts"])
    graph_fts = f32(inputs["graph_fts"])
    adj_mat = np.ascontiguousarray(np.asarray(inputs["adj_mat"], dtype=np.int32))

    shared = {}
    shared["wpack"] = np.ascontiguousarray(np.concatenate(
        [f32(inputs[w]) for w in ("W2", "W1", "Wg", "Wo1", "Wo2")], axis=1
    ))
    shared["bpack"] = np.ascontiguousarray(np.concatenate(
        [f32(inputs[b]).reshape(1, MID)
         for b in ("b1", "b2", "be", "bg", "bo1", "bo2")], axis=1
    ))
    shared["We"] = f32(inputs["We"])

    in_maps = []
    for c in range(NCORES):
        b, ih = c // 2, c % 2
        sl = slice(ih * IH, (ih + 1) * IH)
        m = dict(shared)
        m["edge"] = np.ascontiguousarray(edge_fts[b, :, sl, :])
        m["nodeT"] = np.ascontiguousarray(node_fts[b].T)
        m["noderT"] = np.ascontiguousarray(node_fts[b, sl, :].T)
        m["graph"] = np.ascontiguousarray(graph_fts[b]).reshape(1, D)
        m["adj"] = np.ascontiguousarray(adj_mat[b, :, sl])
        in_maps.append(m)

    res = run_bass_kernel_spmd(nc, in_maps, list(range(NCORES)))

    out = np.empty((B, N, OUT), dtype=np.float32)
    for c in range(NCORES):
        b, ih = c // 2, c % 2
        out[b, ih * IH:(ih + 1) * IH, :] = res.results[c]["out"]
    return out

